# revision 3
# baseline (speedup 1.0000x reference)
"""Bahdanau-attention Bass kernel for 8 TRN2 NeuronCores (data-parallel over batch).

Shapes (hardcoded): B=128, S=1024, EH2=1024, DH=512, A=512.
Returns (context [B, EH2] f32, attn_weights [B, S] f32) matching the reference.

Strategy per core (16 batch rows per core, no cross-core communication):
  - Host ships encoder_outputs in bf16 twice: transposed [e, s] chunks for the
    big proj matmul (PE contracts over partitions) and natural [s, e] chunks for
    the context matmul. All host-side prep is free w.r.t. HW exec time.
  - projT[a, s] = sum_e W_enc[e, a] * encT[e, s]  (64 MMs/row, K-dense, bf16)
  - energyT = tanh(projT + dec_projT[:, row] + b_attnT) via ACT with fused
    per-partition bias (a on partitions).
  - scores[1, s] = sum_a v[a] * energyT[a, s] (8 MMs/row, M=1)
  - expm = exp(scores) * maskf  (no max-subtraction needed: |scores| <= ||v||_1)
  - row->partition transpose of expm via 8 K=1 matmuls against [1,1] ones.
  - ctx[1, e] = sum_s expm[s] * enc[s, e] (16 MMs/row, M=1, accumulate over s)
  - normalize ctx and expm by 1/sum(expm), DMA out [1, 2048] per row.
"""

import os

import numpy as np
import ml_dtypes

B, S, E, DH, A = 128, 1024, 1024, 512, 512
NCORES = 8

LAST_EXEC_NS = None

_NC_CACHE = {}


def _build_nc(rows):
    import concourse.tile as tile
    from concourse import bacc, mybir

    f32 = mybir.dt.float32
    bf16 = mybir.dt.bfloat16
    Tanh = mybir.ActivationFunctionType.Tanh
    Exp = mybir.ActivationFunctionType.Exp
    Ident = mybir.ActivationFunctionType.Identity
    AX = mybir.AxisListType.X

    nc = bacc.Bacc(
        "TRN2", target_bir_lowering=False, debug=False, num_devices=NCORES
    )

    encT_d = nc.declare_dram_parameter("encT", [rows, 128, 8, S], bf16, isOutput=False)
    enc_d = nc.declare_dram_parameter("enc", [rows, 128, 8, E], bf16, isOutput=False)
    w_d = nc.declare_dram_parameter("w", [128, 8, A], bf16, isOutput=False)
    wd_d = nc.declare_dram_parameter("wd", [128, 4, A], bf16, isOutput=False)
    dhT_d = nc.declare_dram_parameter("dhT", [128, 4, rows], bf16, isOutput=False)
    bcols_d = nc.declare_dram_parameter("bcols", [128, 4], f32, isOutput=False)
    vcols_d = nc.declare_dram_parameter("vcols", [128, 4], bf16, isOutput=False)
    maskf_d = nc.declare_dram_parameter("maskf", [1, rows, S], f32, isOutput=False)
    out_d = nc.declare_dram_parameter("out", [rows, E + S], f32, isOutput=True)

    with tile.TileContext(nc) as tc:
        with (
            tc.tile_pool(name="singles", bufs=1) as singles,
            tc.tile_pool(name="enc_pool", bufs=2) as enc_pool,
            tc.tile_pool(name="encT_pool", bufs=2) as encT_pool,
            tc.tile_pool(name="energy_pool", bufs=2) as energy_pool,
            tc.tile_pool(name="small", bufs=3) as small,
            tc.tile_pool(name="outp", bufs=3) as outp,
            tc.tile_pool(name="mmps", bufs=2, space="PSUM") as mmps,
            tc.tile_pool(name="vecps", bufs=2, space="PSUM") as vecps,
        ):
            w_sb = singles.tile([128, 8, A], bf16)
            nc.sync.dma_start(out=w_sb[:], in_=w_d[:])
            wd_sb = singles.tile([128, 4, A], bf16)
            nc.sync.dma_start(out=wd_sb[:], in_=wd_d[:])
            dhT_sb = singles.tile([128, 4, rows], bf16)
            nc.sync.dma_start(out=dhT_sb[:], in_=dhT_d[:])
            bcols_sb = singles.tile([128, 4], f32)
            nc.sync.dma_start(out=bcols_sb[:], in_=bcols_d[:])
            vcols_sb = singles.tile([128, 4], bf16)
            nc.sync.dma_start(out=vcols_sb[:], in_=vcols_d[:])
            maskf_sb = singles.tile([1, rows, S], f32)
            nc.sync.dma_start(out=maskf_sb[:], in_=maskf_d[:])
            one_sb = singles.tile([1, 1], f32)
            nc.vector.memset(one_sb, 1.0)

            # dec_projT[a, r] + b_attnT[a], laid out [128(a%), 4(a-chunk), rows]
            dp_ps = vecps.tile([128, 4 * rows], f32, tag="v")
            for m in range(4):
                for k in range(4):
                    nc.tensor.matmul(
                        dp_ps[:, m * rows : (m + 1) * rows],
                        lhsT=wd_sb[:, k, m * 128 : (m + 1) * 128],
                        rhs=dhT_sb[:, k, :],
                        start=(k == 0),
                        stop=(k == 3),
                    )
            dpT_sb = singles.tile([128, 4, rows], f32)
            for m in range(4):
                nc.scalar.activation(
                    out=dpT_sb[:, m, :],
                    in_=dp_ps[:, m * rows : (m + 1) * rows],
                    func=Ident,
                    bias=bcols_sb[:, m : m + 1],
                    scale=1.0,
                )

            for r in range(rows):
                encT_t = encT_pool.tile([128, 8, S], bf16)
                nc.sync.dma_start(out=encT_t[:], in_=encT_d[r, :, :, :])
                enc_t = enc_pool.tile([128, 8, E], bf16)
                nc.sync.dma_start(out=enc_t[:], in_=enc_d[r, :, :, :])

                # projT -> tanh -> energyT (bf16), a on partitions
                energyT = energy_pool.tile([128, 4, S], bf16)
                for m in range(4):
                    mm = mmps.tile([128, S], mybir.dt.float32, tag="mm")
                    for k in range(8):
                        for n in range(2):
                            nc.tensor.matmul(
                                mm[:, n * 512 : (n + 1) * 512],
                                lhsT=w_sb[:, k, m * 128 : (m + 1) * 128],
                                rhs=encT_t[:, k, n * 512 : (n + 1) * 512],
                                start=(k == 0),
                                stop=(k == 7),
                            )
                    nc.scalar.activation(
                        out=energyT[:, m, :],
                        in_=mm[:],
                        func=Tanh,
                        bias=dpT_sb[:, m, r : r + 1],
                        scale=1.0,
                    )

                # scores flat [1, S]
                scores_ps = vecps.tile([1, S], mybir.dt.float32, tag="v")
                for m in range(4):
                    for n in range(2):
                        nc.tensor.matmul(
                            scores_ps[0:1, n * 512 : (n + 1) * 512],
                            lhsT=vcols_sb[:, m : m + 1],
                            rhs=energyT[:, m, n * 512 : (n + 1) * 512],
                            start=(m == 0),
                            stop=(m == 3),
                        )

                expraw = small.tile([1, S], mybir.dt.float32)
                nc.scalar.activation(out=expraw[:], in_=scores_ps[0:1, :], func=Exp)
                expm = small.tile([1, S], mybir.dt.float32)
                nc.vector.tensor_mul(expm[:], expraw[:], maskf_sb[0:1, r, :])
                ssum = small.tile([1, 1], mybir.dt.float32)
                nc.vector.reduce_sum(out=ssum[:], in_=expm[:], axis=AX)
                recip = small.tile([1, 1], mybir.dt.float32)
                nc.vector.reciprocal(out=recip[:], in_=ssum[:])

                # transpose expm row -> partitions via K=1 matmuls
                expT_ps = vecps.tile([128, 8], mybir.dt.float32, tag="v")
                for c in range(8):
                    nc.tensor.matmul(
                        expT_ps[:, c : c + 1],
                        lhsT=expm[0:1, c * 128 : (c + 1) * 128],
                        rhs=one_sb[:],
                        start=True,
                        stop=True,
                    )
                expT_sb = small.tile([128, 8], bf16)
                nc.vector.tensor_copy(out=expT_sb[:], in_=expT_ps[:])

                # ctx[1, E] accumulated over s-chunks
                ctx_ps = vecps.tile([1, E], mybir.dt.float32, tag="v")
                for eh in range(2):
                    for c in range(8):
                        nc.tensor.matmul(
                            ctx_ps[0:1, eh * 512 : (eh + 1) * 512],
                            lhsT=expT_sb[:, c : c + 1],
                            rhs=enc_t[:, c, eh * 512 : (eh + 1) * 512],
                            start=(c == 0),
                            stop=(c == 7),
                        )

                outrow = outp.tile([1, E + S], mybir.dt.float32)
                nc.vector.tensor_scalar_mul(
                    out=outrow[0:1, 0:E], in0=ctx_ps[0:1, :], scalar1=recip[:]
                )
                nc.vector.tensor_scalar_mul(
                    out=outrow[0:1, E : E + S], in0=expm[:], scalar1=recip[:]
                )
                nc.sync.dma_start(out=out_d[r : r + 1, :], in_=outrow[:])

    nc.compile()
    return nc


def _prep_inputs(encoder_outputs, decoder_hidden, src_mask, W_attn, b_attn, v):
    bf16 = ml_dtypes.bfloat16
    b = encoder_outputs.shape[0]
    rows = b // NCORES

    enc_bf = np.asarray(encoder_outputs, dtype=np.float32).astype(bf16)
    # encT_a[b, p, c, s] = enc[b, s, c*128+p]
    encT_a = np.ascontiguousarray(enc_bf.reshape(b, S, 8, 128).transpose(0, 3, 2, 1))
    # enc_a[b, p, c, e] = enc[b, c*128+p, e]
    enc_a = np.ascontiguousarray(enc_bf.reshape(b, 8, 128, E).transpose(0, 2, 1, 3))

    W = np.asarray(W_attn, dtype=np.float32)
    w_a = np.ascontiguousarray(W[:E].astype(bf16).reshape(8, 128, A).transpose(1, 0, 2))
    wd_a = np.ascontiguousarray(
        W[E:].astype(bf16).reshape(4, 128, A).transpose(1, 0, 2)
    )
    bcols = np.ascontiguousarray(
        np.asarray(b_attn, dtype=np.float32).reshape(4, 128).T
    )
    vcols = np.ascontiguousarray(
        np.asarray(v, dtype=np.float32).astype(bf16).reshape(4, 128).T
    )

    dh = np.asarray(decoder_hidden, dtype=np.float32)
    maskf = (np.asarray(src_mask) != 0).astype(np.float32)

    in_maps = []
    for i in range(NCORES):
        sl = slice(i * rows, (i + 1) * rows)
        dh_sh = dh[sl]  # [rows, DH]
        dhT_a = np.ascontiguousarray(
            dh_sh.T.astype(bf16).reshape(4, 128, rows).transpose(1, 0, 2)
        )
        in_maps.append(
            {
                "encT": encT_a[sl],
                "enc": enc_a[sl],
                "w": w_a,
                "wd": wd_a,
                "dhT": dhT_a,
                "bcols": bcols,
                "vcols": vcols,
                "maskf": np.ascontiguousarray(maskf[sl]).reshape(1, rows, S),
            }
        )
    return in_maps, rows


def kernel(encoder_outputs, decoder_hidden, src_mask, W_attn, b_attn, v):
    global LAST_EXEC_NS
    from concourse.bass_utils import run_bass_kernel_spmd

    in_maps, rows = _prep_inputs(
        encoder_outputs, decoder_hidden, src_mask, W_attn, b_attn, v
    )

    if rows not in _NC_CACHE:
        _NC_CACHE[rows] = _build_nc(rows)
    nc = _NC_CACHE[rows]

    trace = os.environ.get("KERNEL_TRACE", "0") == "1"
    res = run_bass_kernel_spmd(nc, in_maps, core_ids=list(range(NCORES)), trace=trace)
    LAST_EXEC_NS = res.exec_time_ns

    ctx = np.concatenate([r["out"][:, :E] for r in res.results], axis=0)
    attn = np.concatenate([r["out"][:, E:] for r in res.results], axis=0)
    return ctx.astype(np.float32), attn.astype(np.float32)


# revision 9
# speedup vs baseline: 1.0125x; 1.0125x over previous
"""Bahdanau-attention Bass kernel for 8 TRN2 NeuronCores (data-parallel over batch).

Shapes (hardcoded): B=128, S=1024, EH2=1024, DH=512, A=512.
Returns (context [B, EH2] f32, attn_weights [B, S] f32) matching the reference.

Strategy per core (16 batch rows per core, no cross-core communication):
  - Host ships encoder_outputs in bf16 twice: transposed [e, s] chunks for the
    big proj matmul (PE contracts over partitions) and natural [s, e] chunks for
    the context matmul. All host-side prep is free w.r.t. HW exec time.
  - projT[a, s] = sum_e W_enc[e, a] * encT[e, s]  (64 MMs/row, K-dense, bf16)
  - energyT = tanh(projT + dec_projT[:, row] + b_attnT) via ACT with fused
    per-partition bias (a on partitions).
  - scores[1, s] = sum_a v[a] * energyT[a, s] (8 MMs/row, M=1)
  - expm = exp(scores) * maskf  (no max-subtraction needed: |scores| <= ||v||_1)
  - row->partition transpose of expm via 8 K=1 matmuls against [1,1] ones.
  - ctx[1, e] = sum_s expm[s] * enc[s, e] (16 MMs/row, M=1, accumulate over s)
  - normalize ctx and expm by 1/sum(expm), DMA out [1, 2048] per row.
"""

import os

import numpy as np
import ml_dtypes

B, S, E, DH, A = 128, 1024, 1024, 512, 512
NCORES = 8

LAST_EXEC_NS = None

_NC_CACHE = {}


def _build_nc(rows):
    import concourse.tile as tile
    from concourse import bacc, mybir

    f32 = mybir.dt.float32
    bf16 = mybir.dt.bfloat16
    Tanh = mybir.ActivationFunctionType.Tanh
    Exp = mybir.ActivationFunctionType.Exp
    Ident = mybir.ActivationFunctionType.Identity
    AX = mybir.AxisListType.X

    nc = bacc.Bacc(
        "TRN2", target_bir_lowering=False, debug=False, num_devices=NCORES
    )

    encT_d = nc.declare_dram_parameter("encT", [rows, 128, 8, S], bf16, isOutput=False)
    enc_d = nc.declare_dram_parameter("enc", [rows, 128, 8, E], bf16, isOutput=False)
    w_d = nc.declare_dram_parameter("w", [128, 8, A], bf16, isOutput=False)
    wd_d = nc.declare_dram_parameter("wd", [128, 4, A], bf16, isOutput=False)
    dhT_d = nc.declare_dram_parameter("dhT", [128, 4, rows], bf16, isOutput=False)
    bcols_d = nc.declare_dram_parameter("bcols", [128, 4], f32, isOutput=False)
    vcols_d = nc.declare_dram_parameter("vcols", [128, 4], bf16, isOutput=False)
    maskf_d = nc.declare_dram_parameter("maskf", [1, rows, S], f32, isOutput=False)
    out_d = nc.declare_dram_parameter("out", [rows, E + S], f32, isOutput=True)

    with tile.TileContext(nc) as tc:
        with (
            tc.tile_pool(name="singles", bufs=1) as singles,
            tc.tile_pool(name="enc_pool", bufs=2) as enc_pool,
            tc.tile_pool(name="encT_pool", bufs=2) as encT_pool,
            tc.tile_pool(name="energy_pool", bufs=2) as energy_pool,
            tc.tile_pool(name="small", bufs=3) as small,
            tc.tile_pool(name="outp", bufs=3) as outp,
            tc.tile_pool(name="mmps", bufs=4, space="PSUM") as mmps,
            tc.tile_pool(name="vecps", bufs=2, space="PSUM") as vecps,
        ):
            w_sb = singles.tile([128, 8, A], bf16)
            nc.sync.dma_start(out=w_sb[:], in_=w_d[:])
            wd_sb = singles.tile([128, 4, A], bf16)
            nc.sync.dma_start(out=wd_sb[:], in_=wd_d[:])
            dhT_sb = singles.tile([128, 4, rows], bf16)
            nc.sync.dma_start(out=dhT_sb[:], in_=dhT_d[:])
            bcols_sb = singles.tile([128, 4], f32)
            nc.sync.dma_start(out=bcols_sb[:], in_=bcols_d[:])
            vcols_sb = singles.tile([128, 4], bf16)
            nc.sync.dma_start(out=vcols_sb[:], in_=vcols_d[:])
            one_sb = singles.tile([1, 1], f32)
            nc.vector.memset(one_sb, 1.0)

            # dec_projT[a, r] + b_attnT[a], laid out [128(a%), 4(a-chunk), rows]
            dp_ps = vecps.tile([128, 4 * rows], f32, tag="v")
            for m in range(4):
                for k in range(4):
                    nc.tensor.matmul(
                        dp_ps[:, m * rows : (m + 1) * rows],
                        lhsT=wd_sb[:, k, m * 128 : (m + 1) * 128],
                        rhs=dhT_sb[:, k, :],
                        start=(k == 0),
                        stop=(k == 3),
                    )
            dpT_sb = singles.tile([128, 4, rows], f32)
            for m in range(4):
                nc.scalar.activation(
                    out=dpT_sb[:, m, :],
                    in_=dp_ps[:, m * rows : (m + 1) * rows],
                    func=Ident,
                    bias=bcols_sb[:, m : m + 1],
                    scale=1.0,
                )

            for r in range(rows):
                encT_t = encT_pool.tile([128, 8, S], bf16)
                for c in range(8):
                    nc.sync.dma_start(out=encT_t[:, c, :], in_=encT_d[r, :, c, :])
                enc_t = enc_pool.tile([128, 8, E], bf16)
                for c in range(8):
                    nc.sync.dma_start(out=enc_t[:, c, :], in_=enc_d[r, :, c, :])

                # projT -> tanh -> energyT (bf16), a on partitions
                energyT = energy_pool.tile([128, 4, S], bf16)
                for m in range(4):
                    for n in range(2):
                        mm = mmps.tile([128, 512], mybir.dt.float32, tag="mm")
                        for k in range(8):
                            nc.tensor.matmul(
                                mm[:],
                                lhsT=w_sb[:, k, m * 128 : (m + 1) * 128],
                                rhs=encT_t[:, k, n * 512 : (n + 1) * 512],
                                start=(k == 0),
                                stop=(k == 7),
                            )
                        nc.scalar.activation(
                            out=energyT[:, m, n * 512 : (n + 1) * 512],
                            in_=mm[:],
                            func=Tanh,
                            bias=dpT_sb[:, m, r : r + 1],
                            scale=1.0,
                        )

                # scores flat [1, S]
                scores_ps = vecps.tile([1, S], mybir.dt.float32, tag="v")
                for m in range(4):
                    for n in range(2):
                        nc.tensor.matmul(
                            scores_ps[0:1, n * 512 : (n + 1) * 512],
                            lhsT=vcols_sb[:, m : m + 1],
                            rhs=energyT[:, m, n * 512 : (n + 1) * 512],
                            start=(m == 0),
                            stop=(m == 3),
                        )

                maskrow = small.tile([1, S], mybir.dt.float32)
                nc.sync.dma_start(out=maskrow[:], in_=maskf_d[0, r : r + 1, :])
                expraw = small.tile([1, S], mybir.dt.float32)
                nc.scalar.activation(out=expraw[:], in_=scores_ps[0:1, :], func=Exp)
                expm = small.tile([1, S], mybir.dt.float32)
                nc.vector.tensor_mul(expm[:], expraw[:], maskrow[:])
                srtile = small.tile([1, 8], mybir.dt.float32)
                ssum = srtile[0:1, 0:1]
                recip = srtile[0:1, 1:2]
                nc.vector.reduce_sum(out=ssum, in_=expm[:], axis=AX)
                nc.vector.reciprocal(out=recip, in_=ssum)

                # transpose expm row -> partitions via K=1 matmuls
                expT_ps = vecps.tile([128, 8], mybir.dt.float32, tag="v")
                for c in range(8):
                    nc.tensor.matmul(
                        expT_ps[:, c : c + 1],
                        lhsT=expm[0:1, c * 128 : (c + 1) * 128],
                        rhs=one_sb[:],
                        start=True,
                        stop=True,
                    )
                expT_sb = small.tile([128, 8], bf16)
                nc.vector.tensor_copy(out=expT_sb[:], in_=expT_ps[:])

                # ctx[1, E] accumulated over s-chunks
                ctx_ps = vecps.tile([1, E], mybir.dt.float32, tag="v")
                for eh in range(2):
                    for c in range(8):
                        nc.tensor.matmul(
                            ctx_ps[0:1, eh * 512 : (eh + 1) * 512],
                            lhsT=expT_sb[:, c : c + 1],
                            rhs=enc_t[:, c, eh * 512 : (eh + 1) * 512],
                            start=(c == 0),
                            stop=(c == 7),
                        )

                outrow = outp.tile([1, E + S], mybir.dt.float32)
                nc.vector.tensor_scalar_mul(
                    out=outrow[0:1, 0:E], in0=ctx_ps[0:1, :], scalar1=recip
                )
                nc.vector.tensor_scalar_mul(
                    out=outrow[0:1, E : E + S], in0=expm[:], scalar1=recip
                )
                nc.sync.dma_start(out=out_d[r : r + 1, :], in_=outrow[:])

    nc.compile()
    return nc


def _prep_inputs(encoder_outputs, decoder_hidden, src_mask, W_attn, b_attn, v):
    bf16 = ml_dtypes.bfloat16
    b = encoder_outputs.shape[0]
    rows = b // NCORES

    enc_bf = np.asarray(encoder_outputs, dtype=np.float32).astype(bf16)
    # encT_a[b, p, c, s] = enc[b, s, c*128+p]
    encT_a = np.ascontiguousarray(enc_bf.reshape(b, S, 8, 128).transpose(0, 3, 2, 1))
    # enc_a[b, p, c, e] = enc[b, c*128+p, e]
    enc_a = np.ascontiguousarray(enc_bf.reshape(b, 8, 128, E).transpose(0, 2, 1, 3))

    W = np.asarray(W_attn, dtype=np.float32)
    w_a = np.ascontiguousarray(W[:E].astype(bf16).reshape(8, 128, A).transpose(1, 0, 2))
    wd_a = np.ascontiguousarray(
        W[E:].astype(bf16).reshape(4, 128, A).transpose(1, 0, 2)
    )
    bcols = np.ascontiguousarray(
        np.asarray(b_attn, dtype=np.float32).reshape(4, 128).T
    )
    vcols = np.ascontiguousarray(
        np.asarray(v, dtype=np.float32).astype(bf16).reshape(4, 128).T
    )

    dh = np.asarray(decoder_hidden, dtype=np.float32)
    maskf = (np.asarray(src_mask) != 0).astype(np.float32)

    in_maps = []
    for i in range(NCORES):
        sl = slice(i * rows, (i + 1) * rows)
        dh_sh = dh[sl]  # [rows, DH]
        dhT_a = np.ascontiguousarray(
            dh_sh.T.astype(bf16).reshape(4, 128, rows).transpose(1, 0, 2)
        )
        in_maps.append(
            {
                "encT": encT_a[sl],
                "enc": enc_a[sl],
                "w": w_a,
                "wd": wd_a,
                "dhT": dhT_a,
                "bcols": bcols,
                "vcols": vcols,
                "maskf": np.ascontiguousarray(maskf[sl]).reshape(1, rows, S),
            }
        )
    return in_maps, rows


def kernel(encoder_outputs, decoder_hidden, src_mask, W_attn, b_attn, v):
    global LAST_EXEC_NS
    from concourse.bass_utils import run_bass_kernel_spmd

    in_maps, rows = _prep_inputs(
        encoder_outputs, decoder_hidden, src_mask, W_attn, b_attn, v
    )

    if rows not in _NC_CACHE:
        _NC_CACHE[rows] = _build_nc(rows)
    nc = _NC_CACHE[rows]

    trace = os.environ.get("KERNEL_TRACE", "0") == "1"
    res = run_bass_kernel_spmd(nc, in_maps, core_ids=list(range(NCORES)), trace=trace)
    LAST_EXEC_NS = res.exec_time_ns

    ctx = np.concatenate([r["out"][:, :E] for r in res.results], axis=0)
    attn = np.concatenate([r["out"][:, E:] for r in res.results], axis=0)
    return ctx.astype(np.float32), attn.astype(np.float32)


# revision 11
# speedup vs baseline: 1.1189x; 1.1051x over previous
"""Bahdanau-attention Bass kernel for 8 TRN2 NeuronCores (data-parallel over batch).

Shapes (hardcoded): B=128, S=1024, EH2=1024, DH=512, A=512.
Returns (context [B, EH2] f32, attn_weights [B, S] f32) matching the reference.

Strategy per core (16 batch rows per core, no cross-core communication):
  - Host ships encoder_outputs once, in bf16, transposed to [e, s] chunks
    ("encT") — the only layout needed: PE contracts over partitions for proj,
    and the context reduction runs on DVE along the free (s) dim of encT.
  - projT[a, s] = sum_e W_enc[e, a] * encT[e, s]   (64 MMs/row, K-dense, bf16)
  - energyT = tanh(projT + dec_projT[:, row] + b_attnT) via ACT with fused
    per-partition bias (a on partitions).
  - scores[1, s] = sum_a v[a] * energyT[a, s]      (8 MMs/row, M=1)
  - expm = exp(scores) * maskrow   (no max-subtraction: |scores| <= ||v||_1)
  - expb = broadcast expm across partitions via two K=1 PE matmuls (ones ⊗ expm),
    evacuated PSUM->SBUF bf16 on ACT; recip broadcast the same way.
  - ctxT[e-chunk] = sum_s (encT * recip) * expb via DVE scalar_tensor_tensor
    with accum_out — one fused pass per e-chunk, no second enc read from HBM.
  - ctxT [128, 8] -> PE transpose (identity) -> [8, 128] -> DMA out.
"""

import os

import numpy as np
import ml_dtypes

B, S, E, DH, A = 128, 1024, 1024, 512, 512
NCORES = 8

LAST_EXEC_NS = None

_NC_CACHE = {}


def _build_nc(rows):
    import concourse.tile as tile
    from concourse import bacc, mybir

    f32 = mybir.dt.float32
    bf16 = mybir.dt.bfloat16
    Tanh = mybir.ActivationFunctionType.Tanh
    Exp = mybir.ActivationFunctionType.Exp
    Ident = mybir.ActivationFunctionType.Identity
    AX = mybir.AxisListType.X
    MULT = mybir.AluOpType.mult

    nc = bacc.Bacc(
        "TRN2", target_bir_lowering=False, debug=False, num_devices=NCORES
    )

    encT_d = nc.declare_dram_parameter("encT", [rows, 128, 8, S], bf16, isOutput=False)
    w_d = nc.declare_dram_parameter("w", [128, 8, A], bf16, isOutput=False)
    wd_d = nc.declare_dram_parameter("wd", [128, 4, A], bf16, isOutput=False)
    dhT_d = nc.declare_dram_parameter("dhT", [128, 4, rows], bf16, isOutput=False)
    bcols_d = nc.declare_dram_parameter("bcols", [128, 4], f32, isOutput=False)
    vcols_d = nc.declare_dram_parameter("vcols", [128, 4], bf16, isOutput=False)
    maskf_d = nc.declare_dram_parameter("maskf", [rows, S], f32, isOutput=False)
    ident_d = nc.declare_dram_parameter("ident", [128, 128], f32, isOutput=False)
    out_d = nc.declare_dram_parameter("out", [rows, E + S], f32, isOutput=True)

    with tile.TileContext(nc) as tc:
        with (
            tc.tile_pool(name="singles", bufs=1) as singles,
            tc.tile_pool(name="encT_pool", bufs=3) as encT_pool,
            tc.tile_pool(name="energy_pool", bufs=2) as energy_pool,
            tc.tile_pool(name="small", bufs=3) as small,
            tc.tile_pool(name="outp", bufs=3) as outp,
            tc.tile_pool(name="mmps", bufs=4, space="PSUM") as mmps,
            tc.tile_pool(name="vecps", bufs=2, space="PSUM") as vecps,
        ):
            wd_sb = singles.tile([128, 4, A], bf16)
            nc.sync.dma_start(out=wd_sb[:], in_=wd_d[:])
            dhT_sb = singles.tile([128, 4, rows], bf16)
            nc.sync.dma_start(out=dhT_sb[:], in_=dhT_d[:])
            bcols_sb = singles.tile([128, 4], f32)
            nc.sync.dma_start(out=bcols_sb[:], in_=bcols_d[:])
            vcols_sb = singles.tile([128, 4], bf16)
            nc.sync.dma_start(out=vcols_sb[:], in_=vcols_d[:])
            ident_sb = singles.tile([128, 128], f32)
            nc.sync.dma_start(out=ident_sb[:], in_=ident_d[:])
            w_sb = singles.tile([128, 8, A], bf16)
            nc.sync.dma_start(out=w_sb[:], in_=w_d[:])
            ones_row = singles.tile([1, 128], f32)
            nc.vector.memset(ones_row, 1.0)

            # dec_projT[a, r] + b_attnT[a], laid out [128(a%), 4(a-chunk), rows]
            dp_ps = vecps.tile([128, 4 * rows], f32, tag="v")
            for m in range(4):
                for k in range(4):
                    nc.tensor.matmul(
                        dp_ps[:, m * rows : (m + 1) * rows],
                        lhsT=wd_sb[:, k, m * 128 : (m + 1) * 128],
                        rhs=dhT_sb[:, k, :],
                        start=(k == 0),
                        stop=(k == 3),
                    )
            dpT_sb = singles.tile([128, 4, rows], f32)
            for m in range(4):
                nc.scalar.activation(
                    out=dpT_sb[:, m, :],
                    in_=dp_ps[:, m * rows : (m + 1) * rows],
                    func=Ident,
                    bias=bcols_sb[:, m : m + 1],
                    scale=1.0,
                )

            for r in range(rows):
                encT_t = encT_pool.tile([128, 8, S], bf16)
                for c in range(8):
                    nc.sync.dma_start(out=encT_t[:, c, :], in_=encT_d[r, :, c, :])

                # projT -> tanh -> energyT (bf16), a on partitions
                energyT = energy_pool.tile([128, 4, S], bf16)
                for m in range(4):
                    for n in range(2):
                        mm = mmps.tile([128, 512], mybir.dt.float32, tag="mm")
                        for k in range(8):
                            nc.tensor.matmul(
                                mm[:],
                                lhsT=w_sb[:, k, m * 128 : (m + 1) * 128],
                                rhs=encT_t[:, k, n * 512 : (n + 1) * 512],
                                start=(k == 0),
                                stop=(k == 7),
                            )
                        nc.scalar.activation(
                            out=energyT[:, m, n * 512 : (n + 1) * 512],
                            in_=mm[:],
                            func=Tanh,
                            bias=dpT_sb[:, m, r : r + 1],
                            scale=1.0,
                        )

                # scores flat [1, S]
                scores_ps = vecps.tile([1, S], mybir.dt.float32, tag="v")
                for m in range(4):
                    for n in range(2):
                        nc.tensor.matmul(
                            scores_ps[0:1, n * 512 : (n + 1) * 512],
                            lhsT=vcols_sb[:, m : m + 1],
                            rhs=energyT[:, m, n * 512 : (n + 1) * 512],
                            start=(m == 0),
                            stop=(m == 3),
                        )

                maskrow = small.tile([1, S], mybir.dt.float32)
                nc.sync.dma_start(out=maskrow[:], in_=maskf_d[r : r + 1, :])
                expraw = small.tile([1, S], mybir.dt.float32)
                nc.scalar.activation(out=expraw[:], in_=scores_ps[0:1, :], func=Exp)
                expm = small.tile([1, S], mybir.dt.float32)
                nc.vector.tensor_mul(expm[:], expraw[:], maskrow[:])
                srtile = small.tile([1, 8], mybir.dt.float32)
                ssum = srtile[0:1, 0:1]
                recip = srtile[0:1, 1:2]
                nc.vector.reduce_sum(out=ssum, in_=expm[:], axis=AX)
                nc.vector.reciprocal(out=recip, in_=ssum)

                # broadcast expm across partitions: expb[p, s] = expm[s]
                expb_ps = vecps.tile([128, S], mybir.dt.float32, tag="v")
                for n in range(2):
                    nc.tensor.matmul(
                        expb_ps[:, n * 512 : (n + 1) * 512],
                        lhsT=ones_row[:],
                        rhs=expm[0:1, n * 512 : (n + 1) * 512],
                        start=True,
                        stop=True,
                    )
                expb_sb = small.tile([128, S], bf16)
                nc.scalar.copy(out=expb_sb[:], in_=expb_ps[:])

                # broadcast recip across partitions
                recipb_ps = vecps.tile([128, 1], mybir.dt.float32, tag="v")
                nc.tensor.matmul(
                    recipb_ps[:],
                    lhsT=ones_row[:],
                    rhs=recip,
                    start=True,
                    stop=True,
                )
                recipb_sb = small.tile([128, 1], mybir.dt.float32)
                nc.vector.tensor_copy(out=recipb_sb[:], in_=recipb_ps[:])

                # ctxT[e%128, e-chunk] = sum_s encT * recip * expb  (fused DVE)
                ctxT_sb = small.tile([128, 8], mybir.dt.float32)
                scratch = small.tile([128, S], bf16)
                for c in range(8):
                    nc.vector.scalar_tensor_tensor(
                        out=scratch[:],
                        in0=encT_t[:, c, :],
                        scalar=recipb_sb[:],
                        in1=expb_sb[:],
                        op0=MULT,
                        op1=MULT,
                        accum_out=ctxT_sb[:, c : c + 1],
                    )

                # transpose ctxT -> [8, 128] and write out
                ctxt_ps = vecps.tile([8, 128], mybir.dt.float32, tag="v")
                nc.tensor.transpose(ctxt_ps[:], ctxT_sb[:], ident_sb[:])
                outc = outp.tile([8, 1, 128], mybir.dt.float32, tag="outc")
                nc.vector.tensor_copy(out=outc[:, 0, :], in_=ctxt_ps[:])
                nc.sync.dma_start(
                    out=out_d[r : r + 1, 0:E].rearrange("1 (c f) -> c 1 f", c=8),
                    in_=outc[:],
                )

                outa = outp.tile([1, S], mybir.dt.float32, tag="outa")
                nc.vector.tensor_scalar_mul(out=outa[:], in0=expm[:], scalar1=recip)
                nc.sync.dma_start(out=out_d[r : r + 1, E : E + S], in_=outa[:])

    nc.compile()
    return nc


def _prep_inputs(encoder_outputs, decoder_hidden, src_mask, W_attn, b_attn, v):
    bf16 = ml_dtypes.bfloat16
    b = encoder_outputs.shape[0]
    rows = b // NCORES

    enc_bf = np.asarray(encoder_outputs, dtype=np.float32).astype(bf16)
    # encT_a[b, p, c, s] = enc[b, s, c*128+p]
    encT_a = np.ascontiguousarray(enc_bf.reshape(b, S, 8, 128).transpose(0, 3, 2, 1))

    W = np.asarray(W_attn, dtype=np.float32)
    w_a = np.ascontiguousarray(W[:E].astype(bf16).reshape(8, 128, A).transpose(1, 0, 2))
    wd_a = np.ascontiguousarray(
        W[E:].astype(bf16).reshape(4, 128, A).transpose(1, 0, 2)
    )
    bcols = np.ascontiguousarray(
        np.asarray(b_attn, dtype=np.float32).reshape(4, 128).T
    )
    vcols = np.ascontiguousarray(
        np.asarray(v, dtype=np.float32).astype(bf16).reshape(4, 128).T
    )
    ident = np.eye(128, dtype=np.float32)

    dh = np.asarray(decoder_hidden, dtype=np.float32)
    maskf = (np.asarray(src_mask) != 0).astype(np.float32)

    in_maps = []
    for i in range(NCORES):
        sl = slice(i * rows, (i + 1) * rows)
        dh_sh = dh[sl]  # [rows, DH]
        dhT_a = np.ascontiguousarray(
            dh_sh.T.astype(bf16).reshape(4, 128, rows).transpose(1, 0, 2)
        )
        in_maps.append(
            {
                "encT": encT_a[sl],
                "w": w_a,
                "wd": wd_a,
                "dhT": dhT_a,
                "bcols": bcols,
                "vcols": vcols,
                "maskf": np.ascontiguousarray(maskf[sl]),
                "ident": ident,
            }
        )
    return in_maps, rows


def kernel(encoder_outputs, decoder_hidden, src_mask, W_attn, b_attn, v):
    global LAST_EXEC_NS
    from concourse.bass_utils import run_bass_kernel_spmd

    in_maps, rows = _prep_inputs(
        encoder_outputs, decoder_hidden, src_mask, W_attn, b_attn, v
    )

    if rows not in _NC_CACHE:
        _NC_CACHE[rows] = _build_nc(rows)
    nc = _NC_CACHE[rows]

    trace = os.environ.get("KERNEL_TRACE", "0") == "1"
    res = run_bass_kernel_spmd(nc, in_maps, core_ids=list(range(NCORES)), trace=trace)
    LAST_EXEC_NS = res.exec_time_ns

    ctx = np.concatenate([r["out"][:, :E] for r in res.results], axis=0)
    attn = np.concatenate([r["out"][:, E:] for r in res.results], axis=0)
    return ctx.astype(np.float32), attn.astype(np.float32)
